# revision 22
# baseline (speedup 1.0000x reference)
"""Trainium2 Bass kernel for nn_DMS_STGAT (dual-branch GAT attention softmaxes).

Strategy (per core, data-parallel over batch B=16 -> 2 per core):
  The reference reduces to per-column dot products s1/s2/t1/t2 (128-dim,
  against host-precomputed Wa vectors) plus a scrambled-pair gather
  e[m] = LRelu(d1[r1[m]] + d2[r2[m]] (+ const)) and a double softmax
  over n1-groups.

  v2 design (vs the 49µs baseline):
  - dots: tiny wa stationary [128,2] STREAMS X (fp16, 1 cyc/row) instead of
    125-wide stationaries streaming 2 cols; three dot groups (spatial,
    temporal, d2-distance) run on col-groups 0/32/64 of the PE array
    concurrently.
  - the dot matmul rhs uses reordered 4D APs (j-major for spatial+d2,
    t-major for temporal) so every corner-turn DMA is a clean
    [25 part x 100B] block copy (3 DMAs total).
  - both branches share ONE stacked-K fp16 E-matmul [77,100]@[77,625]:
    rows 0:25 Q1, 25:50 Q2, 50:75 qs (spatial pe term), 75:76 qp hi/lo
    (temporal positional constant split into fp16 hi+lo ones-rows).
  - LeakyRelu in one DVE scalar_tensor_tensor op; exp-overflow safety via
    per-n2-group host constant csh subtracted post-LRelu (cancels in
    softmax).
"""
import sys
import numpy as np

for _p in ("/opt/trn_rl_repo", "/root/.axon_site/_ro/trn_rl_repo"):
    if _p not in sys.path:
        sys.path.insert(0, _p)

from contextlib import ExitStack  # noqa: E402

import concourse.bass as bass  # noqa: E402
import concourse.tile as tile  # noqa: E402
from concourse import bacc, mybir  # noqa: E402

B, C, T, J, F = 16, 128, 25, 25, 256
N = 25            # N == T == J
NN = N * N        # 625
NCORES = 8
BL = B // NCORES  # 2 batches per core
NC2 = BL * NN     # 1250 columns per core
FP = mybir.dt.float32
HF = mybir.dt.float16
AF = mybir.ActivationFunctionType
ALU = mybir.AluOpType

KE = 98           # E-matmul K rows: Q1@0:25, Q2@32:57, qs@64:89, qp@96:98
ME = 114          # E-matmul M cols: spatial 0:50, zero 50:64, temporal 64:114

# j/t chunking of the dot matmuls (psum bank = 512 fp32 per partition)
CHUNKS = [(0, 10), (10, 20), (20, 25)]
# m' chunking of the E matmul / softmax tail (groups of 25, n2-major)
MCHUNKS = [(0, 13, 0), (13, 25, 512)]  # (group lo, group hi, psum free off)

# Pin ALL activation functions to one table set (exp/ln/copy/prelu live
# together in natural_log_exp_and_others) so only one ACT_TABLE_LOAD happens.
_orig_get_tables = bacc.get_activation_tables


def _pinned_tables(arch):
    tabs = dict(_orig_get_tables(arch))
    assert "natural_log_exp_and_others" in tabs
    return {k: (v if k == "natural_log_exp_and_others" else set())
            for k, v in tabs.items()}


bacc.get_activation_tables = _pinned_tables

# ---------------------------------------------------------------- host math --

def _pair_indices():
    r1 = np.zeros(NN, np.int64)
    r2 = np.zeros(NN, np.int64)
    for m in range(NN):
        k1, k2 = 2 * m, 2 * m + 1
        r1[m] = (k1 // N) if k1 < NN else ((k1 - NN) % N)
        r2[m] = (k2 // N) if k2 < NN else ((k2 - NN) % N)
    return r1, r2


def _sinusoid_pos():
    pos = np.arange(200)[:, None].astype(np.float64)
    hid = np.arange(C)[None, :]
    angle = pos / np.power(10000.0, 2.0 * (hid // 2) / C)
    tab = angle.copy()
    tab[:, 0::2] = np.sin(angle[:, 0::2])
    tab[:, 1::2] = np.cos(angle[:, 1::2])
    return tab[:T] * 1000.0  # [T, C] float64


_R1, _R2 = _pair_indices()
# m' = n2*25 + n1  (n2-major so softmax n1-groups are contiguous runs)
_MPERM = (np.arange(NN) % N) * N + (np.arange(NN) // N)


def _host_consts(W_s, a_s, W_t, a_t):
    """Precompute tiny derived params in float64. ~0.3 MFLOP."""
    W_s = W_s.astype(np.float64)
    a_s = a_s.astype(np.float64)
    W_t = W_t.astype(np.float64)
    a_t = a_t.astype(np.float64)
    wa_s1 = W_s @ a_s[:F, 0]
    wa_s2 = W_s @ a_s[F:, 0]
    wa_t1 = W_t @ a_t[:F, 0]
    wa_t2 = W_t @ a_t[F:, 0]
    S1, S2 = wa_s1.sum(), wa_s2.sum()

    Q1 = np.zeros((N, NN), np.float64)
    Q2 = np.zeros((N, NN), np.float64)
    Q1[_R1, np.arange(NN)] = 1.0
    Q2[_R2, np.arange(NN)] = 1.0
    qs = S1 * Q1 + S2 * Q2

    pos = _sinusoid_pos()
    p1 = pos @ wa_t1
    p2 = pos @ wa_t2
    qp = (p1[_R1] + p2[_R2])[_MPERM]               # [625] m'-basis
    qp_hi = np.float16(qp).astype(np.float64)
    qp_lo = qp - qp_hi
    qLR = np.where(qp > 0, qp, 0.2 * qp)
    cq = qLR.reshape(N, N).max(axis=1)             # max over n1 per n2 group
    csh = np.repeat(cq, N)                         # [625] m'-basis

    qstk = np.zeros((KE, NN), np.float64)
    qstk[0:N] = Q1[:, _MPERM]
    qstk[32:32 + N] = Q2[:, _MPERM]
    qstk[64:64 + N] = qs[:, _MPERM]
    qstk[96] = qp_hi
    qstk[97] = qp_lo

    w6 = np.zeros((C, 5), np.float64)
    w6[:, 0] = wa_s1
    w6[:, 1] = wa_s2
    w6[:, 2] = wa_t1
    w6[:, 3] = wa_t2
    w6[:, 4] = 1.0

    csh50 = np.tile(csh[None, :], (2 * N, 1))
    return (w6.astype(np.float16), qstk.astype(np.float16),
            csh50.astype(np.float32))


# ------------------------------------------------------------- bass program --

def _build_program():
    nc = bacc.Bacc("TRN2", target_bir_lowering=False, debug=False)

    src_d = nc.dram_tensor("src_l", [BL, C, T, J], FP, kind="ExternalInput").ap()
    w6_d = nc.dram_tensor("w6", [C, 5], HF, kind="ExternalInput").ap()
    qstk_d = nc.dram_tensor("qstk", [KE, NN], HF, kind="ExternalInput").ap()
    csh_d = nc.dram_tensor("csh50", [2 * N, NN], FP, kind="ExternalInput").ap()
    outs_d = nc.dram_tensor("out_s", [BL, T, N, N], FP, kind="ExternalOutput").ap()
    outt_d = nc.dram_tensor("out_t", [BL, T, N, N], FP, kind="ExternalOutput").ap()

    with tile.TileContext(nc) as tc, ExitStack() as ctx:
        consts = ctx.enter_context(tc.tile_pool(name="consts", bufs=1))
        data = ctx.enter_context(tc.tile_pool(name="data", bufs=1))
        pp = ctx.enter_context(tc.tile_pool(name="pp", bufs=1, space="PSUM"))

        X = data.tile([C, NC2], FP)
        X16 = data.tile([C, NC2], HF)
        SUB = data.tile([C, NC2], HF)
        D2 = data.tile([C, NC2], HF)
        W6 = consts.tile([C, 5], HF)
        QS = consts.tile([KE, NN], HF)
        CSH = consts.tile([ME, NN], FP)  # csh50 lives in rows 64:114
        DOT = data.tile([66, NC2], HF)
        LK = data.tile([KE, ME], HF)
        eL = data.tile([89, 50], FP)   # rows 64:89 used (lane-locked with LK)
        eW = data.tile([89, 50], FP)
        eps_b = consts.tile([89, 1], FP)
        E2 = data.tile([ME, NN], FP)
        g = data.tile([ME, NN], FP)
        att1 = data.tile([ME, NN], FP)
        g2 = data.tile([ME, NN], FP)
        Z = data.tile([ME, N], FP)
        Zr = data.tile([ME, N], FP)
        Z2 = data.tile([ME, N], FP)
        Z2r = data.tile([ME, N], FP)
        outF = data.tile([ME, NN], FP)
        dummy = consts.tile([1, 2], FP)

        pd = [pp.tile([66, 512], FP, name=f"pd{k}") for k in range(3)]
        pe_ = pp.tile([ME, 1024], FP)

        FX = X[:].ap[0][0]
        FX16 = X16[:].ap[0][0]
        FSB = SUB[:].ap[0][0]
        FD2 = D2[:].ap[0][0]
        FD = DOT[:].ap[0][0]
        FL = LK[:].ap[0][0]

        # --- input DMAs first (4-way X split; transfers overlap table load) ---
        for b, eng in ((0, nc.sync), (1, nc.scalar)):
            for c0, c1 in ((0, 64), (64, C)):
                src_b = bass.AP(tensor=src_d.tensor,
                                offset=src_d.offset + b * C * NN + c0 * NN,
                                ap=[[NN, c1 - c0], [1, NN]])
                eng.dma_start(X[c0:c1, b * NN:(b + 1) * NN], src_b)
        nc.sync.dma_start(W6[:], w6_d)
        nc.sync.dma_start(QS[:], qstk_d)
        nc.gpsimd.dma_start(CSH[64:ME, :], csh_d)

        # --- LK zeros + qp ones-rows; ACT table warm-up ---
        nc.vector.memset(LK[:], 0.0)
        nc.vector.memset(LK[96:98, 64:ME], 1.0)
        nc.vector.memset(dummy[:], 0.0)
        nc.scalar.activation(dummy[:], dummy[:], AF.Exp)

        # --- PE HAM warm-up: ~3us of tiny dependency-free matmuls ---
        dW = consts.tile([32, 2], HF)
        dX = consts.tile([32, 8], HF)
        dumP = pp.tile([2, 8], FP)
        nc.vector.memset(dW[:], 1.0)
        nc.vector.memset(dX[:], 1.0)
        for _ in range(32):
            nc.tensor.matmul(dumP[0:2, 0:8], dW[:], dX[:], start=True, stop=True)

        # --- cast X -> fp16 (split across engines, balanced by rate) ---
        nc.vector.tensor_copy(X16[:, 0:700], X[:, 0:700])
        nc.scalar.copy(X16[:, 700:1120], X[:, 700:1120])
        nc.gpsimd.tensor_copy(X16[:, 1120:1250], X[:, 1120:1250])

        # --- SUB = X16 - X16[j=8]; D2 = SUB^2 (per j-chunk, DVE) ---
        for j0, j1 in CHUNKS:
            jw = j1 - j0
            sview = bass.AP(tensor=SUB.tensor, offset=SUB.offset + j0,
                            ap=[[FSB, C], [NN, BL], [N, N], [1, jw]])
            xview = bass.AP(tensor=X16.tensor, offset=X16.offset + j0,
                            ap=[[FX16, C], [NN, BL], [N, N], [1, jw]])
            rview = bass.AP(tensor=X16.tensor, offset=X16.offset + 8,
                            ap=[[FX16, C], [NN, BL], [N, N], [0, jw]])
            dview = bass.AP(tensor=D2.tensor, offset=D2.offset + j0,
                            ap=[[FD2, C], [NN, BL], [N, N], [1, jw]])
            nc.vector.tensor_tensor(sview, xview, rview, op=ALU.subtract)
            nc.vector.tensor_tensor(dview, sview, sview, op=ALU.mult)

        # --- dot matmuls: wa stationary, X streams; col-groups 0/32/64.
        # Order A1-A3, C1-C3, B1-B3: the d2 path (C -> EC) is the longest
        # downstream chain, B only feeds the t-corner DMA.
        def rhs_view(t, pitch, c0, cw, jmajor):
            if jmajor:
                ap = [[pitch, C], [1, cw], [NN, BL], [N, N]]
                off = c0
            else:
                ap = [[pitch, C], [N, cw], [NN, BL], [1, N]]
                off = c0 * N
            return bass.AP(tensor=t.tensor, offset=t.offset + off, ap=ap)

        for k, (c0, c1) in enumerate(CHUNKS):
            w = (c1 - c0) * 50
            nc.tensor.matmul(pd[k][0:2, 0:w], W6[:, 0:2],
                             rhs_view(X16, FX16, c0, c1 - c0, True),
                             start=True, stop=True)
        for k, (c0, c1) in enumerate(CHUNKS):
            w = (c1 - c0) * 50
            nc.tensor.matmul(pd[k][64:65, 0:w], W6[:, 4:5],
                             rhs_view(D2, FD2, c0, c1 - c0, True),
                             start=True, stop=True)
        for k, (c0, c1) in enumerate(CHUNKS):
            w = (c1 - c0) * 50
            nc.tensor.matmul(pd[k][32:34, 0:w], W6[:, 2:4],
                             rhs_view(X16, FX16, c0, c1 - c0, False),
                             start=True, stop=True)

        # --- PSUM -> SBUF (cast to fp16) per chunk and row group ---
        def pcopy(eng, k, r0, r1):
            c0, c1 = CHUNKS[k]
            w = (c1 - c0) * 50
            if eng is nc.scalar:
                eng.copy(DOT[r0:r1, c0 * 50:c0 * 50 + w], pd[k][r0:r1, 0:w])
            else:
                eng.tensor_copy(DOT[r0:r1, c0 * 50:c0 * 50 + w],
                                pd[k][r0:r1, 0:w])

        pcopy(nc.vector, 0, 0, 2)    # A copies
        pcopy(nc.vector, 1, 0, 2)
        pcopy(nc.vector, 2, 0, 2)
        pcopy(nc.scalar, 0, 64, 65)  # C (d2) copies
        pcopy(nc.scalar, 1, 64, 65)
        pcopy(nc.scalar, 2, 64, 65)
        pcopy(nc.vector, 0, 32, 34)  # B copies
        pcopy(nc.vector, 1, 32, 34)

        # --- corner-turn DMAs: [25 part x 100B] block copies per plane ---
        def corner(eng, src_row, dst_poff, dst_coff):
            eng.dma_start(
                bass.AP(tensor=LK.tensor,
                        offset=LK.offset + dst_poff * FL + dst_coff,
                        ap=[[FL, N], [1, 50]]),
                bass.AP(tensor=DOT.tensor, offset=DOT.offset + src_row * FD,
                        ap=[[FD, 1], [50, N], [1, 50]]))

        corner(nc.sync, 0, 0, 0)       # s1 -> LK[0:25, 0:50]
        corner(nc.sync, 1, 32, 0)      # s2 -> LK[32:57, 0:50]
        corner(nc.scalar, 64, 64, 0)   # d2 -> LK[64:89, 0:50]
        pcopy(nc.scalar, 2, 32, 34)    # last B copy (fills d2-DMA latency)
        corner(nc.gpsimd, 32, 0, 64)   # t1 -> LK[0:25, 64:114]
        corner(nc.gpsimd, 33, 32, 64)  # t2 -> LK[32:57, 64:114]

        # --- EC = exp(-sqrt(d2)/1000) in-place on LK[64:89, 0:50] ---
        nc.vector.memset(eps_b[:], 6.1035e-05)
        nc.scalar.activation(eL[64:89, :], LK[64:89, 0:50], AF.Ln,
                             bias=eps_b[64:89])
        nc.scalar.activation(eW[64:89, :], eL[64:89, :], AF.Exp, scale=0.5)
        nc.scalar.activation(LK[64:89, 0:50], eW[64:89, :], AF.Exp, scale=-0.001)

        # --- stacked E matmul: [98,114] @ [98,625] -> psum [114, 625] ---
        for glo, ghi, poff in MCHUNKS:
            w = (ghi - glo) * N
            nc.tensor.matmul(pe_[0:ME, poff:poff + w], LK[:, :],
                             QS[:, glo * N:glo * N + w], start=True, stop=True)

        # --- softmax tail (m' is n2-major: n1-groups are contiguous 25s) ---
        def gview(t, glo, ghi):
            fs = t[:].ap[0][0]
            return bass.AP(tensor=t.tensor, offset=t.offset + glo * N,
                           ap=[[fs, ME], [N, ghi - glo], [1, N]])

        def bview(t, glo, ghi):
            fs = t[:].ap[0][0]
            return bass.AP(tensor=t.tensor, offset=t.offset + glo,
                           ap=[[fs, ME], [1, ghi - glo], [0, N]])

        FO = outF[:].ap[0][0]
        for ci, (glo, ghi, poff) in enumerate(MCHUNKS):
            w = (ghi - glo) * N
            lo = glo * N
            pv = pe_[0:ME, poff:poff + w]
            # LeakyRelu in one ACT op (single PSUM input allowed)
            nc.scalar.activation(E2[:, lo:lo + w], pv, AF.Prelu, alpha=0.2)
            engc = nc.vector if ci == 0 else nc.gpsimd
            engc.tensor_tensor(E2[64:ME, lo:lo + w], E2[64:ME, lo:lo + w],
                               CSH[64:ME, lo:lo + w], op=ALU.subtract)
            nc.scalar.activation(g[:, lo:lo + w], E2[:, lo:lo + w], AF.Exp)
            nc.vector.tensor_reduce(Z[:, glo:ghi], gview(g, glo, ghi),
                                    axis=mybir.AxisListType.X, op=ALU.add)
            nc.vector.reciprocal(Zr[:, glo:ghi], Z[:, glo:ghi])
            eng1 = nc.vector if ci == 0 else nc.gpsimd
            eng1.tensor_tensor(gview(att1, glo, ghi), gview(g, glo, ghi),
                               bview(Zr, glo, ghi), op=ALU.mult)
            nc.scalar.activation(g2[:, lo:lo + w], att1[:, lo:lo + w], AF.Exp)
            nc.vector.tensor_reduce(Z2[:, glo:ghi], gview(g2, glo, ghi),
                                    axis=mybir.AxisListType.X, op=ALU.add)
            nc.vector.reciprocal(Z2r[:, glo:ghi], Z2[:, glo:ghi])
            # final scale, writing transposed back to n1-major for output
            oswap = bass.AP(tensor=outF.tensor, offset=outF.offset + glo,
                            ap=[[FO, ME], [1, ghi - glo], [N, N]])
            eng2 = nc.gpsimd if ci == 0 else nc.vector
            eng2.tensor_tensor(oswap, gview(g2, glo, ghi),
                               bview(Z2r, glo, ghi), op=ALU.mult)

        # --- outputs: rows are (b, t)/(b, j) in natural order -> one DMA each
        nc.sync.dma_start(
            bass.AP(tensor=outs_d.tensor, offset=outs_d.offset,
                    ap=[[NN, 50], [1, NN]]),
            bass.AP(tensor=outF.tensor, offset=outF.offset, ap=[[FO, 50], [1, NN]]))
        nc.gpsimd.dma_start(
            bass.AP(tensor=outt_d.tensor, offset=outt_d.offset,
                    ap=[[NN, 50], [1, NN]]),
            bass.AP(tensor=outF.tensor, offset=outF.offset + 64 * FO,
                    ap=[[FO, 50], [1, NN]]))

    nc.compile()
    return nc


_PROGRAM = None


def _get_program():
    global _PROGRAM
    if _PROGRAM is None:
        _PROGRAM = _build_program()
    return _PROGRAM


# ------------------------------------------------------------------ kernel --

def kernel(src, W_s, a_s, W_t, a_t):
    from concourse.bass_utils import run_bass_kernel_spmd

    src = np.ascontiguousarray(np.asarray(src, dtype=np.float32))
    w6, qstk, csh50 = _host_consts(np.asarray(W_s), np.asarray(a_s),
                                   np.asarray(W_t), np.asarray(a_t))
    nc = _get_program()
    in_maps = []
    for c in range(NCORES):
        in_maps.append({
            "src_l": src[c * BL:(c + 1) * BL],
            "w6": w6, "qstk": qstk, "csh50": csh50,
        })
    res = run_bass_kernel_spmd(nc, in_maps, core_ids=list(range(NCORES)))
    out_s = np.concatenate([res.results[c]["out_s"] for c in range(NCORES)], axis=0)
    out_t = np.concatenate([res.results[c]["out_t"] for c in range(NCORES)], axis=0)
    return out_s, out_t


# revision 24
# speedup vs baseline: 1.2527x; 1.2527x over previous
"""Trainium2 Bass kernel for nn_DMS_STGAT (dual-branch GAT attention softmaxes).

Strategy (per core, data-parallel over batch B=16 -> 2 per core):
  The reference reduces to per-column dot products s1/s2/t1/t2 (128-dim,
  against host-precomputed Wa vectors) plus a scrambled-pair gather
  e[m] = LRelu(d1[r1[m]] + d2[r2[m]] (+ const)) and a double softmax
  over n1-groups.

  v2 design (vs the 49µs baseline):
  - dots: tiny wa stationary [128,2] STREAMS X (fp16, 1 cyc/row) instead of
    125-wide stationaries streaming 2 cols; three dot groups (spatial,
    temporal, d2-distance) run on col-groups 0/32/64 of the PE array
    concurrently.
  - the dot matmul rhs uses reordered 4D APs (j-major for spatial+d2,
    t-major for temporal) so every corner-turn DMA is a clean
    [25 part x 100B] block copy (3 DMAs total).
  - both branches share ONE stacked-K fp16 E-matmul [77,100]@[77,625]:
    rows 0:25 Q1, 25:50 Q2, 50:75 qs (spatial pe term), 75:76 qp hi/lo
    (temporal positional constant split into fp16 hi+lo ones-rows).
  - LeakyRelu in one DVE scalar_tensor_tensor op; exp-overflow safety via
    per-n2-group host constant csh subtracted post-LRelu (cancels in
    softmax).
"""
import sys
import numpy as np

for _p in ("/opt/trn_rl_repo", "/root/.axon_site/_ro/trn_rl_repo"):
    if _p not in sys.path:
        sys.path.insert(0, _p)

from contextlib import ExitStack  # noqa: E402

import concourse.bass as bass  # noqa: E402
import concourse.tile as tile  # noqa: E402
from concourse import bacc, mybir  # noqa: E402

B, C, T, J, F = 16, 128, 25, 25, 256
N = 25            # N == T == J
NN = N * N        # 625
NCORES = 8
BL = B // NCORES  # 2 batches per core
NC2 = BL * NN     # 1250 columns per core
FP = mybir.dt.float32
HF = mybir.dt.float16
AF = mybir.ActivationFunctionType
ALU = mybir.AluOpType

KE = 98           # E-matmul K rows: Q1@0:25, Q2@32:57, qs@64:89, qp@96:98
ME = 114          # E-matmul M cols: spatial 0:50, zero 50:64, temporal 64:114

# j/t chunking of the dot matmuls (psum bank = 512 fp32 per partition)
CHUNKS = [(0, 10), (10, 20), (20, 25)]
# m' chunking of the E matmul / softmax tail (groups of 25, n2-major)
MCHUNKS = [(0, 13, 0), (13, 25, 512)]  # (group lo, group hi, psum free off)

# Pin ALL activation functions to one table set (exp/ln/copy/prelu live
# together in natural_log_exp_and_others) so only one ACT_TABLE_LOAD happens.
_orig_get_tables = bacc.get_activation_tables


def _pinned_tables(arch):
    tabs = dict(_orig_get_tables(arch))
    assert "natural_log_exp_and_others" in tabs
    return {k: (v if k == "natural_log_exp_and_others" else set())
            for k, v in tabs.items()}


bacc.get_activation_tables = _pinned_tables

# ---------------------------------------------------------------- host math --

def _pair_indices():
    r1 = np.zeros(NN, np.int64)
    r2 = np.zeros(NN, np.int64)
    for m in range(NN):
        k1, k2 = 2 * m, 2 * m + 1
        r1[m] = (k1 // N) if k1 < NN else ((k1 - NN) % N)
        r2[m] = (k2 // N) if k2 < NN else ((k2 - NN) % N)
    return r1, r2


def _sinusoid_pos():
    pos = np.arange(200)[:, None].astype(np.float64)
    hid = np.arange(C)[None, :]
    angle = pos / np.power(10000.0, 2.0 * (hid // 2) / C)
    tab = angle.copy()
    tab[:, 0::2] = np.sin(angle[:, 0::2])
    tab[:, 1::2] = np.cos(angle[:, 1::2])
    return tab[:T] * 1000.0  # [T, C] float64


_R1, _R2 = _pair_indices()
# m' = n2*25 + n1  (n2-major so softmax n1-groups are contiguous runs)
_MPERM = (np.arange(NN) % N) * N + (np.arange(NN) // N)


def _host_consts(W_s, a_s, W_t, a_t):
    """Precompute tiny derived params in float64. ~0.3 MFLOP."""
    W_s = W_s.astype(np.float64)
    a_s = a_s.astype(np.float64)
    W_t = W_t.astype(np.float64)
    a_t = a_t.astype(np.float64)
    wa_s1 = W_s @ a_s[:F, 0]
    wa_s2 = W_s @ a_s[F:, 0]
    wa_t1 = W_t @ a_t[:F, 0]
    wa_t2 = W_t @ a_t[F:, 0]
    S1, S2 = wa_s1.sum(), wa_s2.sum()

    Q1 = np.zeros((N, NN), np.float64)
    Q2 = np.zeros((N, NN), np.float64)
    Q1[_R1, np.arange(NN)] = 1.0
    Q2[_R2, np.arange(NN)] = 1.0
    qs = S1 * Q1 + S2 * Q2

    pos = _sinusoid_pos()
    p1 = pos @ wa_t1
    p2 = pos @ wa_t2
    qp = (p1[_R1] + p2[_R2])[_MPERM]               # [625] m'-basis
    qp_hi = np.float16(qp).astype(np.float64)
    qp_lo = qp - qp_hi
    qLR = np.where(qp > 0, qp, 0.2 * qp)
    cq = qLR.reshape(N, N).max(axis=1)             # max over n1 per n2 group
    csh = np.repeat(cq, N)                         # [625] m'-basis

    qstk = np.zeros((KE, NN), np.float64)
    qstk[0:N] = Q1[:, _MPERM]
    qstk[32:32 + N] = Q2[:, _MPERM]
    qstk[64:64 + N] = qs[:, _MPERM]
    qstk[96] = qp_hi
    qstk[97] = qp_lo

    w6 = np.zeros((C, 5), np.float64)
    w6[:, 0] = wa_s1
    w6[:, 1] = wa_s2
    w6[:, 2] = wa_t1
    w6[:, 3] = wa_t2
    w6[:, 4] = 1.0

    csh50 = np.tile(csh[None, :], (2 * N, 1))
    return (w6.astype(np.float16), qstk.astype(np.float16),
            csh50.astype(np.float32))


# ------------------------------------------------------------- bass program --

def _build_program():
    nc = bacc.Bacc("TRN2", target_bir_lowering=False, debug=False)

    src_d = nc.dram_tensor("src_l", [BL, C, T, J], FP, kind="ExternalInput").ap()
    w6_d = nc.dram_tensor("w6", [C, 5], HF, kind="ExternalInput").ap()
    qstk_d = nc.dram_tensor("qstk", [KE, NN], HF, kind="ExternalInput").ap()
    csh_d = nc.dram_tensor("csh50", [2 * N, NN], FP, kind="ExternalInput").ap()
    outs_d = nc.dram_tensor("out_s", [BL, T, N, N], FP, kind="ExternalOutput").ap()
    outt_d = nc.dram_tensor("out_t", [BL, T, N, N], FP, kind="ExternalOutput").ap()

    with tile.TileContext(nc) as tc, ExitStack() as ctx:
        consts = ctx.enter_context(tc.tile_pool(name="consts", bufs=1))
        data = ctx.enter_context(tc.tile_pool(name="data", bufs=1))
        pp = ctx.enter_context(tc.tile_pool(name="pp", bufs=1, space="PSUM"))

        X = data.tile([C, NC2], FP)
        X16 = data.tile([C, NC2], HF)
        SUB = data.tile([C, NC2], HF)
        D2 = data.tile([C, NC2], HF)
        W6 = consts.tile([C, 5], HF)
        QS = consts.tile([KE, NN], HF)
        CSH = consts.tile([ME, NN], FP)  # csh50 lives in rows 64:114
        DOT = data.tile([66, NC2], HF)
        LK = data.tile([KE, ME], HF)
        eL = data.tile([89, 50], FP)   # rows 64:89 used (lane-locked with LK)
        eW = data.tile([89, 50], FP)
        eps_b = consts.tile([89, 1], FP)
        E2 = data.tile([ME, NN], FP)
        g = data.tile([ME, NN], FP)
        att1 = data.tile([ME, NN], FP)
        g2 = data.tile([ME, NN], FP)
        Z = data.tile([ME, N], FP)
        Zr = data.tile([ME, N], FP)
        Z2 = data.tile([ME, N], FP)
        Z2r = data.tile([ME, N], FP)
        outF = data.tile([ME, NN], FP)
        dummy = consts.tile([1, 2], FP)

        pd = [pp.tile([66, 512], FP, name=f"pd{k}") for k in range(3)]
        pe_ = pp.tile([ME, 1024], FP)

        FX = X[:].ap[0][0]
        FX16 = X16[:].ap[0][0]
        FSB = SUB[:].ap[0][0]
        FD2 = D2[:].ap[0][0]
        FD = DOT[:].ap[0][0]
        FL = LK[:].ap[0][0]

        # --- input DMAs first. Keep the scalar ring clear for Xb1 so both X
        # halves land ~3.3us after issue; consts ride sync/gpsimd rings.
        for b, eng in ((0, nc.sync), (1, nc.scalar)):
            src_b = bass.AP(tensor=src_d.tensor, offset=src_d.offset + b * C * NN,
                            ap=[[NN, C], [1, NN]])
            eng.dma_start(X[:, b * NN:(b + 1) * NN], src_b)
        nc.sync.dma_start(W6[:], w6_d)
        nc.sync.dma_start(QS[:], qstk_d)
        nc.gpsimd.dma_start(CSH[64:ME, :], csh_d)

        # --- LK zeros + qp ones-rows; ACT table warm-up ---
        nc.vector.memset(LK[:], 0.0)
        nc.vector.memset(LK[96:98, 64:ME], 1.0)
        nc.vector.memset(dummy[:], 0.0)
        nc.scalar.activation(dummy[:], dummy[:], AF.Exp)

        # --- PE HAM warm-up: ~5us of junk matmuls ending as X lands, so the
        # real dots run at 2.4GHz instead of the cold 0.65-1.2GHz pstate.
        dW = consts.tile([32, 2], HF)
        dJ = data.tile([32, 512], HF)
        dumP = pp.tile([2, 512], FP)
        nc.vector.memset(dW[:], 1.0)
        nc.vector.memset(dJ[:], 0.0)
        for _ in range(12):
            nc.tensor.matmul(dumP[0:2, 0:512], dW[:], dJ[:],
                             start=True, stop=True)

        # --- cast X -> fp16 (split on the b boundary: [0:625] only needs Xb0)
        nc.vector.tensor_copy(X16[:, 0:625], X[:, 0:625])
        nc.scalar.copy(X16[:, 625:1050], X[:, 625:1050])
        nc.gpsimd.tensor_copy(X16[:, 1050:1250], X[:, 1050:1250])

        # --- SUB = X - X[j=8] (fp32 src: starts at Xb1 arrival, before casts
        # finish); D2 = SUB^2. High priority pins the chain early on DVE.
        with tc.high_priority():
            for j0, j1 in CHUNKS:
                jw = j1 - j0
                sview = bass.AP(tensor=SUB.tensor, offset=SUB.offset + j0,
                                ap=[[FSB, C], [NN, BL], [N, N], [1, jw]])
                xview = bass.AP(tensor=X.tensor, offset=X.offset + j0,
                                ap=[[FX, C], [NN, BL], [N, N], [1, jw]])
                rview = bass.AP(tensor=X.tensor, offset=X.offset + 8,
                                ap=[[FX, C], [NN, BL], [N, N], [0, jw]])
                dview = bass.AP(tensor=D2.tensor, offset=D2.offset + j0,
                                ap=[[FD2, C], [NN, BL], [N, N], [1, jw]])
                nc.vector.tensor_tensor(sview, xview, rview, op=ALU.subtract)
                nc.vector.tensor_tensor(dview, sview, sview, op=ALU.mult)

        # --- dot matmuls: wa stationary, X streams; col-groups 0/32/64 run
        # concurrently (A=q0, B=q32, C=q64), one trio per psum chunk.
        def rhs_view(t, pitch, c0, cw, jmajor):
            if jmajor:
                ap = [[pitch, C], [1, cw], [NN, BL], [N, N]]
                off = c0
            else:
                ap = [[pitch, C], [N, cw], [NN, BL], [1, N]]
                off = c0 * N
            return bass.AP(tensor=t.tensor, offset=t.offset + off, ap=ap)

        for k, (c0, c1) in enumerate(CHUNKS):
            cw = c1 - c0
            w = cw * 50
            nc.tensor.matmul(pd[k][64:65, 0:w], W6[:, 4:5],
                             rhs_view(D2, FD2, c0, cw, True),
                             start=True, stop=True)
            nc.tensor.matmul(pd[k][0:2, 0:w], W6[:, 0:2],
                             rhs_view(X16, FX16, c0, cw, True),
                             start=True, stop=True)
            nc.tensor.matmul(pd[k][32:34, 0:w], W6[:, 2:4],
                             rhs_view(X16, FX16, c0, cw, False),
                             start=True, stop=True)

        # --- PSUM -> SBUF (cast to fp16) per chunk ---
        for k, (c0, c1) in enumerate(CHUNKS):
            w = (c1 - c0) * 50
            eng = (nc.vector, nc.scalar, nc.vector)[k]
            if eng is nc.scalar:
                eng.copy(DOT[0:66, c0 * 50:c0 * 50 + w], pd[k][0:66, 0:w])
            else:
                eng.tensor_copy(DOT[0:66, c0 * 50:c0 * 50 + w], pd[k][0:66, 0:w])

        # --- corner-turn DMAs: [25 part x 100B] block copies per plane ---
        def corner(eng, src_row, dst_poff, dst_coff):
            eng.dma_start(
                bass.AP(tensor=LK.tensor,
                        offset=LK.offset + dst_poff * FL + dst_coff,
                        ap=[[FL, N], [1, 50]]),
                bass.AP(tensor=DOT.tensor, offset=DOT.offset + src_row * FD,
                        ap=[[FD, 1], [50, N], [1, 50]]))

        corner(nc.scalar, 64, 64, 0)   # d2 -> LK[64:89, 0:50] (longest chain)
        corner(nc.sync, 0, 0, 0)       # s1 -> LK[0:25, 0:50]
        corner(nc.sync, 1, 32, 0)      # s2 -> LK[32:57, 0:50]
        corner(nc.gpsimd, 32, 0, 64)   # t1 -> LK[0:25, 64:114]
        corner(nc.gpsimd, 33, 32, 64)  # t2 -> LK[32:57, 64:114]

        # --- EC = exp(-sqrt(d2)/1000) in-place on LK[64:89, 0:50] ---
        nc.vector.memset(eps_b[:], 6.1035e-05)
        nc.scalar.activation(eL[64:89, :], LK[64:89, 0:50], AF.Ln,
                             bias=eps_b[64:89])
        nc.scalar.activation(eW[64:89, :], eL[64:89, :], AF.Exp, scale=0.5)
        nc.scalar.activation(LK[64:89, 0:50], eW[64:89, :], AF.Exp, scale=-0.001)

        # --- stacked E matmul: [98,114] @ [98,625] -> psum [114, 625] ---
        for glo, ghi, poff in MCHUNKS:
            w = (ghi - glo) * N
            nc.tensor.matmul(pe_[0:ME, poff:poff + w], LK[:, :],
                             QS[:, glo * N:glo * N + w], start=True, stop=True)

        # --- softmax tail (m' is n2-major: n1-groups are contiguous 25s) ---
        def gview(t, glo, ghi):
            fs = t[:].ap[0][0]
            return bass.AP(tensor=t.tensor, offset=t.offset + glo * N,
                           ap=[[fs, ME], [N, ghi - glo], [1, N]])

        def bview(t, glo, ghi):
            fs = t[:].ap[0][0]
            return bass.AP(tensor=t.tensor, offset=t.offset + glo,
                           ap=[[fs, ME], [1, ghi - glo], [0, N]])

        FO = outF[:].ap[0][0]
        for ci, (glo, ghi, poff) in enumerate(MCHUNKS):
            w = (ghi - glo) * N
            lo = glo * N
            pv = pe_[0:ME, poff:poff + w]
            # LeakyRelu in one ACT op (single PSUM input allowed)
            nc.scalar.activation(E2[:, lo:lo + w], pv, AF.Prelu, alpha=0.2)
            engc = nc.vector if ci == 0 else nc.gpsimd
            engc.tensor_tensor(E2[64:ME, lo:lo + w], E2[64:ME, lo:lo + w],
                               CSH[64:ME, lo:lo + w], op=ALU.subtract)
            nc.scalar.activation(g[:, lo:lo + w], E2[:, lo:lo + w], AF.Exp)
            nc.vector.tensor_reduce(Z[:, glo:ghi], gview(g, glo, ghi),
                                    axis=mybir.AxisListType.X, op=ALU.add)
            nc.vector.reciprocal(Zr[:, glo:ghi], Z[:, glo:ghi])
            eng1 = nc.vector if ci == 0 else nc.gpsimd
            eng1.tensor_tensor(gview(att1, glo, ghi), gview(g, glo, ghi),
                               bview(Zr, glo, ghi), op=ALU.mult)
            nc.scalar.activation(g2[:, lo:lo + w], att1[:, lo:lo + w], AF.Exp)
            nc.vector.tensor_reduce(Z2[:, glo:ghi], gview(g2, glo, ghi),
                                    axis=mybir.AxisListType.X, op=ALU.add)
            nc.vector.reciprocal(Z2r[:, glo:ghi], Z2[:, glo:ghi])
            # final scale, writing transposed back to n1-major for output
            oswap = bass.AP(tensor=outF.tensor, offset=outF.offset + glo,
                            ap=[[FO, ME], [1, ghi - glo], [N, N]])
            eng2 = nc.gpsimd if ci == 0 else nc.vector
            eng2.tensor_tensor(oswap, gview(g2, glo, ghi),
                               bview(Z2r, glo, ghi), op=ALU.mult)

        # --- outputs: rows are (b, t)/(b, j) in natural order -> one DMA each
        nc.sync.dma_start(
            bass.AP(tensor=outs_d.tensor, offset=outs_d.offset,
                    ap=[[NN, 50], [1, NN]]),
            bass.AP(tensor=outF.tensor, offset=outF.offset, ap=[[FO, 50], [1, NN]]))
        nc.gpsimd.dma_start(
            bass.AP(tensor=outt_d.tensor, offset=outt_d.offset,
                    ap=[[NN, 50], [1, NN]]),
            bass.AP(tensor=outF.tensor, offset=outF.offset + 64 * FO,
                    ap=[[FO, 50], [1, NN]]))

    nc.compile()
    return nc


_PROGRAM = None


def _get_program():
    global _PROGRAM
    if _PROGRAM is None:
        _PROGRAM = _build_program()
    return _PROGRAM


# ------------------------------------------------------------------ kernel --

def kernel(src, W_s, a_s, W_t, a_t):
    from concourse.bass_utils import run_bass_kernel_spmd

    src = np.ascontiguousarray(np.asarray(src, dtype=np.float32))
    w6, qstk, csh50 = _host_consts(np.asarray(W_s), np.asarray(a_s),
                                   np.asarray(W_t), np.asarray(a_t))
    nc = _get_program()
    in_maps = []
    for c in range(NCORES):
        in_maps.append({
            "src_l": src[c * BL:(c + 1) * BL],
            "w6": w6, "qstk": qstk, "csh50": csh50,
        })
    res = run_bass_kernel_spmd(nc, in_maps, core_ids=list(range(NCORES)))
    out_s = np.concatenate([res.results[c]["out_s"] for c in range(NCORES)], axis=0)
    out_t = np.concatenate([res.results[c]["out_t"] for c in range(NCORES)], axis=0)
    return out_s, out_t
